# revision 9
# baseline (speedup 1.0000x reference)
"""DeepFM forward on 8 Trainium2 NeuronCores.

Data-parallel: batch 8192 -> 1024 samples/core; tables replicated.

Math (weight-only preprocessing on host):
  logit_b = fm_b + wide_b + deep_b + b_ffn
  A = sym(w2)/2 = V diag(lam) V^T (float64 eigh); E_b = emb[x_b] [TS, F]
  P_b = V^T [E_b*32 | H_b*2048]  (fp8 rhs, one matmul per 4 samples)
  fm_b   = sum_k lam_k/32^2 sum_f P_b[k, f<64]^2
  wide_b + deep_b = <[V^T W3/32 | V^T W1/2048], P_b>_F   (DVE mult+reduce)

Gather strategy: the problem is a per-core random gather of 102400
(sample, slot) rows x 96 B. SWDGE desc-gen runs ~7.8 ns/desc per queue
with 4 queues concurrent, so the kernel is descriptor-generation bound:
  stage 1: per chunk of 256 samples, gather the ~5.7k unique rows per
    25000-row vocab window (4 windows -> 4 balanced queues, 96 B elems,
    -1 tail truncates each list at its true length) into SBUF staging,
    then HWDGE-writeback into a compact [24576 row, 256 B-stride] HBM
    table per chunk.
  stage 2: dma_gather 6400 rows per 64-sample sub from the compact
    table (ids < 24576 fit int16) in slot-major order -- NO pad slots.
    The gather's wrap order (p=i%128, r=i//128) -> slot order fix-up is
    affine, so two cheap HWDGE hops (SBUF->HBM slotbuf in wrap order,
    HBM->SBUF re-read in [t-partition, sample] order) land
    sel[t, sample, 96B] for the matmul without SWDGE descriptors.
Software pipeline (depth 3): all index tiles prefetched at t=0;
stage-1 of chunk c+2 is emitted BEFORE stage-2 of chunk c so the
in-order Pool sequencer keeps all 4 queues generating continuously.
  dma_gather needs single_packet=False (>=64 descs/lane crashes the
  device otherwise); 96 B elems at 256 B stride need raw
  InstDMAGatherAnt construction (bass's %256 elem assert is
  transpose-only in ucode).
"""

import os
import numpy as np

import concourse.bass as bass
import concourse.mybir as mybir
from concourse import bacc, ap_utils
from concourse.tile import TileContext
from concourse.bass_utils import run_bass_kernel_spmd

BS, TS, VOCAB, F = 8192, 100, 100000, 64
K = 32
NCORES = 8
SPC = BS // NCORES        # 1024 samples per core
EMB_SCALE = 32.0
H_SCALE = 2048.0

WINW = 25000              # vocab rows per window (balanced across queues)
NBANK = 4
BROWS = 32768             # big-table rows per window (aligned region)
ROWB = 256                # big/compact table row stride (bytes)
PAYB = 96                 # used bytes per row: 64 fp8 emb + 32 fp8 h

CHS = 256                 # samples per chunk
NCH = SPC // CHS          # 4 chunks
M1 = 6144                 # stage-1 list length per window per chunk
CRANK = M1 // 128         # staging ranks per window
CROWS = NBANK * M1        # 24576 compact rows per chunk
CB = [k * M1 for k in range(NBANK)]
M1TOT16 = CROWS // 16     # i1 columns per chunk
SUB = 64                  # stage-2 samples per sub-gather (6400 idxs)
NSUB = CHS // SUB         # 4 sub-gathers per chunk
NSLOT = SUB * TS          # 6400 slots per sub
GRP = 4                   # samples per matmul (384 PSUM cols, 1 bank)
SUPER = 8                 # samples per PSUM super-tile (2 matmul groups)

U8 = mybir.dt.uint8
FP8 = mybir.dt.float8e4
BF16 = mybir.dt.bfloat16
F32 = mybir.dt.float32
I16 = mybir.dt.int16

_cached = {}


def _raw_gather(g, out_ap, in_ap, idxs_ap, num_idxs, elem_size, queue_num):
    """dma_gather minus the elem_size%256 assert (non-transpose, HBM src)."""
    assert idxs_ap.dtype == I16
    elem_step = in_ap.ap[0][0]
    stride_bytes = elem_step * mybir.dt.size(in_ap.dtype)
    assert stride_bytes % 256 == 0
    assert ap_utils.ap_is_contiguous(in_ap.ap[1:])
    assert ap_utils.ap_is_contiguous(out_ap.ap[1:])
    assert ap_utils.ap_is_contiguous(idxs_ap.ap[1:])
    assert in_ap.ap[-1][1] == out_ap.ap[-1][1] == elem_size
    _in = g.lower_ap_dma(in_ap, for_custom_bir_dma=True)
    return g.add_instruction(
        mybir.InstDMAGatherAnt(
            name=g.bass.get_next_instruction_name(),
            ins=[*_in, g.lower_ap(idxs_ap),
                 g.lower_val_access(g.to_reg(num_idxs))],
            outs=[g.lower_ap(out_ap)],
            transpose=False, num_idxs=num_idxs, elem_size=elem_size,
            stride_bytes_256=stride_bytes // 256, gen_mode=0,
            single_packet=False, queue_num=queue_num,
            sbuf_tokens_per_rank=0, sbuf_free_dim_per_rank=0,
            sbuf_free_dim_pad_per_rank=0, sbuf_byte_offset=0,
        ))


def build_nc():
    nc = bacc.Bacc("TRN2", target_bir_lowering=False, debug=False,
                   num_devices=NCORES, num_swdge_queues=NBANK)
    btab = nc.dram_tensor("btab", [NBANK * BROWS, ROWB], U8,
                          kind="ExternalInput")
    i1 = nc.dram_tensor("i1", [NCH, 128, M1TOT16], I16,
                        kind="ExternalInput")
    i2 = nc.dram_tensor("i2", [NCH, NSUB, 128, NSLOT // 16], I16,
                        kind="ExternalInput")
    vmat = nc.dram_tensor("vmat", [128, TS], BF16, kind="ExternalInput")
    lam = nc.dram_tensor("lam", [TS, 1], F32, kind="ExternalInput")
    onesv = nc.dram_tensor("onesv", [128, 1], F32, kind="ExternalInput")
    linp = nc.dram_tensor("linp", [128, PAYB], BF16, kind="ExternalInput")
    bffn = nc.dram_tensor("bffn", [1, 1], F32, kind="ExternalInput")
    ctabs = [nc.dram_tensor(f"ctab{c}", [CROWS, ROWB], U8, kind="Internal")
             for c in range(NCH)]
    slotb = [[nc.dram_tensor(f"slot{c}_{s}", [NSLOT, PAYB], U8,
                             kind="Internal")
              for s in range(NSUB)] for c in range(NCH)]
    y = nc.dram_tensor("y", [1, SPC], F32, kind="ExternalOutput")

    with TileContext(nc) as tc:
        with (
            tc.tile_pool(name="const", bufs=1) as cpool,
            tc.tile_pool(name="acc", bufs=1) as apool,
            tc.tile_pool(name="stg", bufs=2) as spool,
            tc.tile_pool(name="tmp", bufs=4) as tpool,
            tc.tile_pool(name="sel", bufs=6) as lpool,
            tc.tile_pool(name="sq", bufs=2) as qpool,
            tc.tile_pool(name="psum", bufs=2, space="PSUM") as ppool,
            tc.tile_pool(name="psuml", bufs=1, space="PSUM") as plpool,
        ):
            # index tiles first so chunk-0 stage-1 can start ASAP
            i1_sb = cpool.tile([128, NCH, M1TOT16], I16)
            nc.sync.dma_start(out=i1_sb[:],
                              in_=i1.ap().rearrange("c p n -> p c n"))
            i2_sb = cpool.tile([128, NCH, NSUB, NSLOT // 16], I16)
            nc.sync.dma_start(out=i2_sb[:],
                              in_=i2.ap().rearrange("c s p n -> p c s n"))
            v_sb = cpool.tile([128, TS], BF16)
            nc.sync.dma_start(out=v_sb[:], in_=vmat.ap())
            lam_sb = cpool.tile([TS, 1], F32)
            nc.sync.dma_start(out=lam_sb[:], in_=lam.ap())
            ones_sb = cpool.tile([128, 1], F32)
            nc.sync.dma_start(out=ones_sb[:], in_=onesv.ap())
            lin_sb = cpool.tile([128, PAYB], BF16)
            nc.sync.dma_start(out=lin_sb[:], in_=linp.ap())
            bffn_sb = cpool.tile([1, 1], F32)
            nc.sync.dma_start(out=bffn_sb[:], in_=bffn.ap())

            acc_sq = apool.tile([TS, SPC], F32)
            acc_lin = apool.tile([TS, SPC], F32)

            def emit_stage1(c):
                # 256 B elems so the writeback is one contiguous 12 KB run
                # per partition per window (HWDGE descriptor count matters).
                stg = spool.tile([128, NBANK * CRANK, ROWB], U8, tag="stg")
                for k in range(NBANK):
                    r0 = k * CRANK
                    _raw_gather(
                        nc.gpsimd, stg[:, r0:r0 + CRANK, :],
                        btab.ap()[k * BROWS:(k + 1) * BROWS, :],
                        i1_sb[:, c, CB[k] // 16:(CB[k] + M1) // 16],
                        M1, ROWB, queue_num=k)
                    nc.sync.dma_start(
                        out=ctabs[c].ap()[CB[k]:CB[k] + M1, :]
                        .rearrange("(p r) e -> p r e", p=128),
                        in_=stg[:, r0:r0 + CRANK, :])

            def emit_stage2(c):
                sels = []
                for s in range(NSUB):
                    tmp = tpool.tile([128, NSLOT // 128, PAYB], U8, tag="tmp")
                    _raw_gather(nc.gpsimd, tmp[:],
                                ctabs[c].ap()[:, 0:PAYB],
                                i2_sb[:, c, s, :], NSLOT, PAYB,
                                queue_num=s)
                    # slots enumerated t-major (i = t*SUB + b), so hop 1
                    # scatters 96 B runs (p=i%128, r=i//128 -> row i) and
                    # hop 2 reads one contiguous 6 KB run per t-partition.
                    e1, e2 = ((nc.scalar, nc.sync) if s % 2 == 0
                              else (nc.sync, nc.scalar))
                    e1.dma_start(
                        out=slotb[c][s].ap()
                        .rearrange("(r p) e -> p r e", p=128),
                        in_=tmp[:])
                    sel = lpool.tile([TS, SUB, PAYB], U8, tag="sel")
                    e2.dma_start(
                        out=sel[:],
                        in_=slotb[c][s].ap()
                        .rearrange("(t b) e -> t b e", b=SUB))
                    sels.append(sel)
                return sels

            def emit_compute(c, sels):
                for s in range(NSUB):
                    sel8 = sels[s][:].bitcast(FP8)
                    ng = SUPER // GRP
                    for t in range(SUB // SUPER):
                        p = ppool.tile([TS, ng, 512], F32,
                                       space="PSUM", tag="p")
                        for g in range(ng):
                            nc.tensor.matmul(
                                out=p[:, g, 0:GRP * PAYB],
                                lhsT=v_sb[0:TS],
                                rhs=sel8[:, t * SUPER + g * GRP:
                                         t * SUPER + (g + 1) * GRP, :],
                                start=True, stop=True)
                        base = c * CHS + s * SUB + t * SUPER
                        pv = p[:, :, 0:GRP * PAYB].rearrange(
                            "p g (b e) -> p g b e", e=PAYB)
                        sq = qpool.tile([TS, ng, GRP, F], BF16, tag="sq")
                        nc.scalar.activation(
                            sq[:], pv[:, :, :, 0:F],
                            mybir.ActivationFunctionType.Square)
                        nc.vector.tensor_reduce(
                            out=acc_sq[:, base:base + SUPER].rearrange(
                                "p (g b) -> p g b", g=ng),
                            in_=sq[:],
                            axis=mybir.AxisListType.X, op=mybir.AluOpType.add)
                        lin = qpool.tile([TS, ng, GRP, PAYB], BF16, tag="lin")
                        nc.vector.tensor_tensor(
                            out=lin[:], in0=pv,
                            in1=lin_sb[0:TS]
                            .rearrange("p (a b e) -> p a b e", a=1, b=1)
                            .to_broadcast([TS, ng, GRP, PAYB]),
                            op=mybir.AluOpType.mult)
                        nc.vector.tensor_reduce(
                            out=acc_lin[:, base:base + SUPER].rearrange(
                                "p (g b) -> p g b", g=ng),
                            in_=lin[:],
                            axis=mybir.AxisListType.X, op=mybir.AluOpType.add)

            # software pipeline, depth 3: stage-1 runs two chunks ahead so
            # stage-2 never waits on a writeback at the Pool queue head.
            emit_stage1(0)
            for c in range(NCH):
                if c + 1 < NCH:
                    emit_stage1(c + 1)
                sels = emit_stage2(c)
                emit_compute(c, sels)

            pl = plpool.tile([1, SPC], F32, space="PSUM")
            for h in range((SPC + 511) // 512):
                sl = slice(h * 512, min((h + 1) * 512, SPC))
                nc.tensor.matmul(out=pl[:, sl], lhsT=lam_sb[:],
                                 rhs=acc_sq[:, sl], start=True, stop=False)
                nc.tensor.matmul(out=pl[:, sl], lhsT=ones_sb[0:TS],
                                 rhs=acc_lin[:, sl], start=False, stop=True)
            y_sb = cpool.tile([1, SPC], F32)
            nc.scalar.activation(y_sb[:], pl[:],
                                 mybir.ActivationFunctionType.Sigmoid,
                                 bias=bffn_sb[:, :])
            nc.sync.dma_start(out=y.ap(), in_=y_sb[:])

    nc.compile()
    return nc


def _wrap16(flat):
    """[N] int16 list -> [128, N//16] wrapped+replicated index tile."""
    n = flat.shape[0]
    w = flat.reshape(n // 16, 16).T
    return np.tile(w, (8, 1)).astype(np.int16)


def _host_prep(x, emb, w_deep, b_deep, w_ffn, b_ffn):
    x = np.asarray(x)
    emb = np.asarray(emb, dtype=np.float32)
    w_deep = np.asarray(w_deep, dtype=np.float32)
    b_deep = np.asarray(b_deep, dtype=np.float32)
    w_ffn = np.asarray(w_ffn, dtype=np.float32).reshape(-1)
    b_ffn = np.asarray(b_ffn, dtype=np.float32).reshape(-1)

    n_deep = TS * K
    n_fm = TS * (TS - 1) // 2
    w1 = w_ffn[:n_deep].reshape(TS, K)
    w2 = w_ffn[n_deep:n_deep + n_fm].astype(np.float64)
    w3 = w_ffn[n_deep + n_fm:].reshape(TS, F)

    iu, ju = np.triu_indices(TS, k=1)
    A = np.zeros((TS, TS), dtype=np.float64)
    A[iu, ju] = w2 / 2
    A = A + A.T
    lam, V = np.linalg.eigh(A)

    fp8_np = mybir.dt.np(FP8)
    bf16_np = mybir.dt.np(BF16)

    emb8 = (emb * EMB_SCALE).astype(fp8_np)                        # [V, 64]
    hfeat = (np.maximum(emb.astype(np.float64) @ w_deep + b_deep, 0.0)
             * H_SCALE).astype(fp8_np)                             # [V, 32]
    btab = np.zeros((NBANK * BROWS, ROWB), dtype=np.uint8)
    for k in range(NBANK):
        lo = k * WINW
        n = min(WINW, VOCAB - lo)
        if n <= 0:
            break
        rows = slice(k * BROWS, k * BROWS + n)
        btab[rows, 0:F] = emb8[lo:lo + n].view(np.uint8)
        btab[rows, F:PAYB] = hfeat[lo:lo + n].view(np.uint8)

    vz = np.zeros((128, TS), dtype=bf16_np)
    vz[:TS, :] = V.astype(bf16_np)
    lam_dev = (lam / (EMB_SCALE * EMB_SCALE)).astype(np.float32).reshape(TS, 1)
    onesz = np.zeros((128, 1), dtype=np.float32)
    onesz[:TS] = 1.0
    w3t = (V.T @ w3) / EMB_SCALE                                   # [TS, 64]
    w1t = (V.T @ w1) / H_SCALE                                     # [TS, 32]
    linp = np.zeros((128, PAYB), dtype=bf16_np)
    linp[:TS, 0:F] = w3t.astype(bf16_np)
    linp[:TS, F:PAYB] = w1t.astype(bf16_np)

    shared = {
        "btab": btab, "vmat": vz, "lam": lam_dev, "onesv": onesz,
        "linp": linp, "bffn": b_ffn.reshape(1, 1).astype(np.float32),
    }

    xi = x.astype(np.int64)
    in_maps = []
    for core in range(NCORES):
        xs = xi[core * SPC:(core + 1) * SPC]                       # [SPC, TS]
        i1 = np.zeros((NCH, 128, M1TOT16), dtype=np.int16)
        i2 = np.zeros((NCH, NSUB, 128, NSLOT // 16), dtype=np.int16)
        for c in range(NCH):
            xc = xs[c * CHS:(c + 1) * CHS]                         # [CHS, TS]
            bank = xc // WINW
            local = xc - bank * WINW                               # [CHS, TS]
            cid = np.zeros((CHS, TS), dtype=np.int64)
            for k in range(NBANK):
                msk = bank == k
                uniq = np.unique(local[msk])
                assert len(uniq) <= M1, len(uniq)
                lst = np.full(M1, WINW, dtype=np.int64)     # pad -> zero row
                lst[:len(uniq)] = uniq
                i1[c, :, CB[k] // 16:(CB[k] + M1) // 16] = _wrap16(
                    lst.astype(np.int16))
                # list position i lands at staging (p=i%128, r=i//128),
                # written back to compact row CB[k] + p*CRANK + r
                pos = np.arange(len(uniq))
                lut = np.zeros(WINW, dtype=np.int64)
                lut[uniq] = CB[k] + (pos % 128) * CRANK + pos // 128
                cid[msk] = lut[local[msk]]
            for s in range(NSUB):
                i2[c, s] = _wrap16(
                    cid[s * SUB:(s + 1) * SUB].T.reshape(-1).astype(np.int16))
        in_maps.append({"i1": i1, "i2": i2, **shared})
    return in_maps


def kernel(x, emb, w_deep, b_deep, w_ffn, b_ffn):
    if "nc" not in _cached:
        _cached["nc"] = build_nc()
    nc = _cached["nc"]
    in_maps = _host_prep(x, emb, w_deep, b_deep, w_ffn, b_ffn)
    trace = os.environ.get("KERNEL_TRACE", "") == "1"
    res = run_bass_kernel_spmd(nc, in_maps, core_ids=list(range(NCORES)),
                               trace=trace)
    if trace and res.exec_time_ns is not None:
        print(f"HW exec time: {res.exec_time_ns} ns")
        print(f"mean exec time: {res.mean_exec_time_ns} ns")
        if res.instructions_and_trace:
            print(f"trace: {res.instructions_and_trace[1]}")
    out = np.concatenate([res.results[c]["y"].reshape(SPC)
                          for c in range(NCORES)])
    return out.reshape(BS, 1).astype(np.float32)


# revision 10
# speedup vs baseline: 1.0420x; 1.0420x over previous
"""DeepFM forward on 8 Trainium2 NeuronCores.

Data-parallel: batch 8192 -> 1024 samples/core; tables replicated.

Math (weight-only preprocessing on host):
  logit_b = fm_b + wide_b + deep_b + b_ffn
  A = sym(w2)/2 = V diag(lam) V^T (float64 eigh); E_b = emb[x_b] [TS, F]
  P_b = V^T [E_b*32 | H_b*2048]  (fp8 rhs, one matmul per 4 samples)
  fm_b   = sum_k lam_k/32^2 sum_f P_b[k, f<64]^2
  wide_b + deep_b = <[V^T W3/32 | V^T W1/2048], P_b>_F   (DVE mult+reduce)

Gather strategy: the problem is a per-core random fetch of 102400
(sample, slot) rows x 96 B from a 100k-row table. SWDGE desc-gen runs
~7.8 ns/desc per queue, 4 queues concurrent, so total SWDGE descriptor
count is the roofline. Per sub-batch of 64 samples (6400 slots):
  1. 4 window-gathers (vocab split into 4x 25000-row windows so local
     ids fit int16; window k -> queue k) fetch the sub's slots in
     t-major order, compacted per window, into one staging tile.
  2. ONE dma_scatter_add (queue s%4, indices = each staged slot's
     t*64+b position) permutes staging into a zeroed per-sub HBM slot
     table ([6400 rows, 256 B stride], adds land on zeros; list pads
     gather the window zero-row and scatter +0 onto spread rows).
  3. One contiguous HWDGE read (100 descs x 16 KB) lands
     sel[t-partition, sample, 256B]; the matmul rhs strides 96 of 256.
No compact table, no writeback, no cross-sub barriers: each queue
streams gather gens every sub plus one merged scatter every 4th sub.
  dma_gather needs single_packet=False (>=64 descs/lane crashes the
  device otherwise); 96 B elems at 256 B stride need raw
  InstDMAGatherAnt construction (bass's %256 elem assert is
  transpose-only in ucode). -1 list tails require a matching runtime
  count register, so pads use the window zero-row instead.
"""

import os
import numpy as np

import concourse.bass as bass
import concourse.mybir as mybir
from concourse import bacc, ap_utils
from concourse.tile import TileContext
from concourse.bass_utils import run_bass_kernel_spmd

BS, TS, VOCAB, F = 8192, 100, 100000, 64
K = 32
NCORES = 8
SPC = BS // NCORES        # 1024 samples per core
EMB_SCALE = 32.0
H_SCALE = 2048.0

WINW = 25000              # vocab rows per window (balanced across queues)
NBANK = 4
BROWS = 32768             # big-table rows per window (aligned region)
ROWB = 256                # table row stride (bytes)
PAYB = 96                 # used bytes per row: 64 fp8 emb + 32 fp8 h

SUB = 64                  # samples per sub-batch
NSUBT = SPC // SUB        # 16 sub-batches
NSLOT = SUB * TS          # 6400 slots per sub
M2 = 1792                 # gather list length per window per sub
WRANK = M2 // 128         # staging ranks per window
NIDX = NBANK * M2         # 7168 merged scatter indices per sub
GRP = 4                   # samples per matmul (384 PSUM cols, 1 bank)
SUPER = 8                 # samples per PSUM super-tile (2 matmul groups)

U8 = mybir.dt.uint8
FP8 = mybir.dt.float8e4
BF16 = mybir.dt.bfloat16
F32 = mybir.dt.float32
I16 = mybir.dt.int16

_cached = {}


def _raw_gather(g, out_ap, in_ap, idxs_ap, num_idxs, elem_size, queue_num):
    """dma_gather minus the elem_size%256 assert (non-transpose, HBM src)."""
    assert idxs_ap.dtype == I16
    elem_step = in_ap.ap[0][0]
    stride_bytes = elem_step * mybir.dt.size(in_ap.dtype)
    assert stride_bytes % 256 == 0
    assert ap_utils.ap_is_contiguous(in_ap.ap[1:])
    assert ap_utils.ap_is_contiguous(out_ap.ap[1:])
    assert ap_utils.ap_is_contiguous(idxs_ap.ap[1:])
    assert in_ap.ap[-1][1] == out_ap.ap[-1][1] == elem_size
    _in = g.lower_ap_dma(in_ap, for_custom_bir_dma=True)
    return g.add_instruction(
        mybir.InstDMAGatherAnt(
            name=g.bass.get_next_instruction_name(),
            ins=[*_in, g.lower_ap(idxs_ap),
                 g.lower_val_access(g.to_reg(num_idxs))],
            outs=[g.lower_ap(out_ap)],
            transpose=False, num_idxs=num_idxs, elem_size=elem_size,
            stride_bytes_256=stride_bytes // 256, gen_mode=0,
            single_packet=False, queue_num=queue_num,
            sbuf_tokens_per_rank=0, sbuf_free_dim_per_rank=0,
            sbuf_free_dim_pad_per_rank=0, sbuf_byte_offset=0,
        ))


def build_nc():
    nc = bacc.Bacc("TRN2", target_bir_lowering=False, debug=False,
                   num_devices=NCORES, num_swdge_queues=NBANK)
    btab = nc.dram_tensor("btab", [NBANK * BROWS, ROWB], U8,
                          kind="ExternalInput")
    ig = nc.dram_tensor("ig", [NSUBT, 128, NIDX // 16], I16,
                        kind="ExternalInput")
    isc = nc.dram_tensor("isc", [NSUBT, 128, NIDX // 16], I16,
                         kind="ExternalInput")
    vmat = nc.dram_tensor("vmat", [128, TS], BF16, kind="ExternalInput")
    lam = nc.dram_tensor("lam", [TS, 1], F32, kind="ExternalInput")
    onesv = nc.dram_tensor("onesv", [128, 1], F32, kind="ExternalInput")
    linp = nc.dram_tensor("linp", [128, PAYB], BF16, kind="ExternalInput")
    bffn = nc.dram_tensor("bffn", [1, 1], F32, kind="ExternalInput")
    slotb = [nc.dram_tensor(f"slot{s}", [NSLOT, ROWB], U8, kind="Internal")
             for s in range(NSUBT)]
    y = nc.dram_tensor("y", [1, SPC], F32, kind="ExternalOutput")

    with TileContext(nc) as tc:
        with (
            tc.tile_pool(name="const", bufs=1) as cpool,
            tc.tile_pool(name="acc", bufs=1) as apool,
            tc.tile_pool(name="stg", bufs=4) as spool,
            tc.tile_pool(name="sel", bufs=4) as lpool,
            tc.tile_pool(name="sq", bufs=2) as qpool,
            tc.tile_pool(name="psum", bufs=2, space="PSUM") as ppool,
            tc.tile_pool(name="psuml", bufs=1, space="PSUM") as plpool,
        ):
            # index tiles first so sub-0 gathers can start ASAP
            ig_sb = cpool.tile([128, NSUBT, NIDX // 16], I16)
            nc.sync.dma_start(out=ig_sb[:],
                              in_=ig.ap().rearrange("s p n -> p s n"))
            isc_sb = cpool.tile([128, NSUBT, NIDX // 16], I16)
            nc.sync.dma_start(out=isc_sb[:],
                              in_=isc.ap().rearrange("s p n -> p s n"))
            v_sb = cpool.tile([128, TS], BF16)
            nc.sync.dma_start(out=v_sb[:], in_=vmat.ap())
            lam_sb = cpool.tile([TS, 1], F32)
            nc.sync.dma_start(out=lam_sb[:], in_=lam.ap())
            ones_sb = cpool.tile([128, 1], F32)
            nc.sync.dma_start(out=ones_sb[:], in_=onesv.ap())
            lin_sb = cpool.tile([128, PAYB], BF16)
            nc.sync.dma_start(out=lin_sb[:], in_=linp.ap())
            bffn_sb = cpool.tile([1, 1], F32)
            nc.sync.dma_start(out=bffn_sb[:], in_=bffn.ap())
            zer_sb = cpool.tile([128, NSLOT // 128, ROWB], U8)
            nc.vector.memset(zer_sb[:], 0.0)

            acc_sq = apool.tile([TS, SPC], F32)
            acc_lin = apool.tile([TS, SPC], F32)

            # slot tables must be zero before the scatter-adds land
            for s in range(NSUBT):
                nc.sync.dma_start(
                    out=slotb[s].ap().rearrange("(p r) e -> p r e", p=128),
                    in_=zer_sb[:])

            def emit_gathers(s):
                stg = spool.tile([128, NBANK * WRANK, PAYB], U8, tag="stg")
                for k in range(NBANK):
                    r0 = k * WRANK
                    _raw_gather(
                        nc.gpsimd, stg[:, r0:r0 + WRANK, :],
                        btab.ap()[k * BROWS:(k + 1) * BROWS, 0:PAYB],
                        ig_sb[:, s, k * M2 // 16:(k + 1) * M2 // 16],
                        M2, PAYB, queue_num=k)
                return stg

            def emit_scatter(s, stg):
                nc.gpsimd.dma_scatter_add(
                    out_ap=slotb[s].ap()[:, 0:PAYB],
                    in_ap=stg[:],
                    idxs_ap=isc_sb[:, s, :],
                    num_idxs=NIDX, num_idxs_reg=NIDX,
                    elem_size=PAYB, elem_step=ROWB,
                    single_packet=False, queue_num=s % NBANK)

            def emit_tail(s):
                sel = lpool.tile([TS, SUB, ROWB], U8, tag="sel")
                nc.scalar.dma_start(
                    out=sel[:],
                    in_=slotb[s].ap().rearrange("(t b) e -> t b e", b=SUB))
                sel8 = sel[:].bitcast(FP8)
                ng = SUPER // GRP
                for t in range(SUB // SUPER):
                    p = ppool.tile([TS, ng, 512], F32, space="PSUM", tag="p")
                    for g in range(ng):
                        nc.tensor.matmul(
                            out=p[:, g, 0:GRP * PAYB],
                            lhsT=v_sb[0:TS],
                            rhs=sel8[:, t * SUPER + g * GRP:
                                     t * SUPER + (g + 1) * GRP, 0:PAYB],
                            start=True, stop=True)
                    base = s * SUB + t * SUPER
                    pv = p[:, :, 0:GRP * PAYB].rearrange(
                        "p g (b e) -> p g b e", e=PAYB)
                    sq = qpool.tile([TS, ng, GRP, F], BF16, tag="sq")
                    nc.scalar.activation(
                        sq[:], pv[:, :, :, 0:F],
                        mybir.ActivationFunctionType.Square)
                    nc.vector.tensor_reduce(
                        out=acc_sq[:, base:base + SUPER].rearrange(
                            "p (g b) -> p g b", g=ng),
                        in_=sq[:],
                        axis=mybir.AxisListType.X, op=mybir.AluOpType.add)
                    lin = qpool.tile([TS, ng, GRP, PAYB], BF16, tag="lin")
                    nc.vector.tensor_tensor(
                        out=lin[:], in0=pv,
                        in1=lin_sb[0:TS]
                        .rearrange("p (a b e) -> p a b e", a=1, b=1)
                        .to_broadcast([TS, ng, GRP, PAYB]),
                        op=mybir.AluOpType.mult)
                    nc.vector.tensor_reduce(
                        out=acc_lin[:, base:base + SUPER].rearrange(
                            "p (g b) -> p g b", g=ng),
                        in_=lin[:],
                        axis=mybir.AxisListType.X, op=mybir.AluOpType.add)

            # software pipeline: scatter of sub s is emitted after the
            # gathers of sub s+2, so its staging drains are long done and
            # the Pool sequencer never parks at the queue head.
            stgs = {}
            for s in range(NSUBT):
                stgs[s] = emit_gathers(s)
                if s >= 2:
                    emit_scatter(s - 2, stgs.pop(s - 2))
                    emit_tail(s - 2)
            for s in (NSUBT - 2, NSUBT - 1):
                emit_scatter(s, stgs.pop(s))
                emit_tail(s)

            pl = plpool.tile([1, SPC], F32, space="PSUM")
            for h in range((SPC + 511) // 512):
                sl = slice(h * 512, min((h + 1) * 512, SPC))
                nc.tensor.matmul(out=pl[:, sl], lhsT=lam_sb[:],
                                 rhs=acc_sq[:, sl], start=True, stop=False)
                nc.tensor.matmul(out=pl[:, sl], lhsT=ones_sb[0:TS],
                                 rhs=acc_lin[:, sl], start=False, stop=True)
            y_sb = cpool.tile([1, SPC], F32)
            nc.scalar.activation(y_sb[:], pl[:],
                                 mybir.ActivationFunctionType.Sigmoid,
                                 bias=bffn_sb[:, :])
            nc.sync.dma_start(out=y.ap(), in_=y_sb[:])

    nc.compile()
    return nc


def _wrap16(flat):
    """[N] int16 list -> [128, N//16] wrapped+replicated index tile."""
    n = flat.shape[0]
    w = flat.reshape(n // 16, 16).T
    return np.tile(w, (8, 1)).astype(np.int16)


def _host_prep(x, emb, w_deep, b_deep, w_ffn, b_ffn):
    x = np.asarray(x)
    emb = np.asarray(emb, dtype=np.float32)
    w_deep = np.asarray(w_deep, dtype=np.float32)
    b_deep = np.asarray(b_deep, dtype=np.float32)
    w_ffn = np.asarray(w_ffn, dtype=np.float32).reshape(-1)
    b_ffn = np.asarray(b_ffn, dtype=np.float32).reshape(-1)

    n_deep = TS * K
    n_fm = TS * (TS - 1) // 2
    w1 = w_ffn[:n_deep].reshape(TS, K)
    w2 = w_ffn[n_deep:n_deep + n_fm].astype(np.float64)
    w3 = w_ffn[n_deep + n_fm:].reshape(TS, F)

    iu, ju = np.triu_indices(TS, k=1)
    A = np.zeros((TS, TS), dtype=np.float64)
    A[iu, ju] = w2 / 2
    A = A + A.T
    lam, V = np.linalg.eigh(A)

    fp8_np = mybir.dt.np(FP8)
    bf16_np = mybir.dt.np(BF16)

    emb8 = (emb * EMB_SCALE).astype(fp8_np)                        # [V, 64]
    hfeat = (np.maximum(emb.astype(np.float64) @ w_deep + b_deep, 0.0)
             * H_SCALE).astype(fp8_np)                             # [V, 32]
    btab = np.zeros((NBANK * BROWS, ROWB), dtype=np.uint8)
    for k in range(NBANK):
        lo = k * WINW
        n = min(WINW, VOCAB - lo)
        if n <= 0:
            break
        rows = slice(k * BROWS, k * BROWS + n)
        btab[rows, 0:F] = emb8[lo:lo + n].view(np.uint8)
        btab[rows, F:PAYB] = hfeat[lo:lo + n].view(np.uint8)

    vz = np.zeros((128, TS), dtype=bf16_np)
    vz[:TS, :] = V.astype(bf16_np)
    lam_dev = (lam / (EMB_SCALE * EMB_SCALE)).astype(np.float32).reshape(TS, 1)
    onesz = np.zeros((128, 1), dtype=np.float32)
    onesz[:TS] = 1.0
    w3t = (V.T @ w3) / EMB_SCALE                                   # [TS, 64]
    w1t = (V.T @ w1) / H_SCALE                                     # [TS, 32]
    linp = np.zeros((128, PAYB), dtype=bf16_np)
    linp[:TS, 0:F] = w3t.astype(bf16_np)
    linp[:TS, F:PAYB] = w1t.astype(bf16_np)

    shared = {
        "btab": btab, "vmat": vz, "lam": lam_dev, "onesv": onesz,
        "linp": linp, "bffn": b_ffn.reshape(1, 1).astype(np.float32),
    }

    xi = x.astype(np.int64)
    in_maps = []
    for core in range(NCORES):
        xs = xi[core * SPC:(core + 1) * SPC]                       # [SPC, TS]
        ig = np.zeros((NSUBT, 128, NIDX // 16), dtype=np.int16)
        isc = np.zeros((NSUBT, 128, NIDX // 16), dtype=np.int16)
        for s in range(NSUBT):
            xc = xs[s * SUB:(s + 1) * SUB]                         # [SUB, TS]
            win = (xc // WINW).T.reshape(-1)     # t-major [TS*SUB]
            loc = (xc - (xc // WINW) * WINW).T.reshape(-1)
            pos = np.arange(NSLOT)               # t-major slot id t*SUB+b
            gl = np.empty(NIDX, dtype=np.int64)
            sl = np.empty(NIDX, dtype=np.int64)
            for k in range(NBANK):
                msk = win == k
                n = int(msk.sum())
                assert n <= M2, n
                glk = np.full(M2, WINW, dtype=np.int64)  # pad -> zero row
                glk[:n] = loc[msk]
                # pads scatter +0 onto spread rows (avoid one-row pileup)
                slk = (np.arange(M2) * 7) % NSLOT
                slk[:n] = pos[msk]
                gl[k * M2:(k + 1) * M2] = glk
                sl[k * M2:(k + 1) * M2] = slk
            ig[s] = _wrap16(gl.astype(np.int16))
            isc[s] = _wrap16(sl.astype(np.int16))
        in_maps.append({"ig": ig, "isc": isc, **shared})
    return in_maps


def kernel(x, emb, w_deep, b_deep, w_ffn, b_ffn):
    if "nc" not in _cached:
        _cached["nc"] = build_nc()
    nc = _cached["nc"]
    in_maps = _host_prep(x, emb, w_deep, b_deep, w_ffn, b_ffn)
    trace = os.environ.get("KERNEL_TRACE", "") == "1"
    res = run_bass_kernel_spmd(nc, in_maps, core_ids=list(range(NCORES)),
                               trace=trace)
    if trace and res.exec_time_ns is not None:
        print(f"HW exec time: {res.exec_time_ns} ns")
        print(f"mean exec time: {res.mean_exec_time_ns} ns")
        if res.instructions_and_trace:
            print(f"trace: {res.instructions_and_trace[1]}")
    out = np.concatenate([res.results[c]["y"].reshape(SPC)
                          for c in range(NCORES)])
    return out.reshape(BS, 1).astype(np.float32)


# revision 15
# speedup vs baseline: 1.0464x; 1.0042x over previous
"""DeepFM forward on 8 Trainium2 NeuronCores.

Data-parallel: batch 8192 -> 1024 samples/core; tables replicated.

Math (weight-only preprocessing on host):
  logit_b = fm_b + wide_b + deep_b + b_ffn
  A = sym(w2)/2 = V diag(lam) V^T (float64 eigh); E_b = emb[x_b] [TS, F]
  P_b = V^T [E_b*32 | H_b*2048]  (fp8 rhs, one matmul per 4 samples)
  fm_b   = sum_k lam_k/32^2 sum_f P_b[k, f<64]^2
  wide_b + deep_b = <[V^T W3/32 | V^T W1/2048], P_b>_F   (DVE mult+reduce)

Gather strategy: the problem is a per-core random fetch of 102400
(sample, slot) rows x 96 B from a 100k-row table. SWDGE desc-gen runs
~7.8 ns/desc per queue, 4 queues concurrent, so total SWDGE descriptor
count is the roofline. Per sub-batch of 64 samples (6400 slots):
  1. 4 window-gathers (vocab split into 4x 25000-row windows so local
     ids fit int16; window k -> queue k) fetch the sub's slots in
     t-major order, compacted per window, into one staging tile.
  2. ONE dma_scatter_add (queue s%4, indices = each staged slot's
     t*64+b position) permutes staging into a zeroed per-sub HBM slot
     table ([6400 rows, 256 B stride], adds land on zeros; list pads
     gather the window zero-row and scatter +0 onto spread rows).
  3. One contiguous HWDGE read (100 descs x 16 KB) lands
     sel[t-partition, sample, 256B]; the matmul rhs strides 96 of 256.
No compact table, no writeback, no cross-sub barriers: each queue
streams gather gens every sub plus one merged scatter every 4th sub.
  dma_gather needs single_packet=False (>=64 descs/lane crashes the
  device otherwise); 96 B elems at 256 B stride need raw
  InstDMAGatherAnt construction (bass's %256 elem assert is
  transpose-only in ucode). -1 list tails require a matching runtime
  count register, so pads use the window zero-row instead.
"""

import os
import numpy as np

import concourse.bass as bass
import concourse.mybir as mybir
from concourse import bacc, ap_utils
from concourse.tile import TileContext
from concourse.bass_utils import run_bass_kernel_spmd

BS, TS, VOCAB, F = 8192, 100, 100000, 64
K = 32
NCORES = 8
SPC = BS // NCORES        # 1024 samples per core
EMB_SCALE = 32.0
H_SCALE = 2048.0

WINW = 25000              # vocab rows per window (balanced across queues)
NBANK = 4
BROWS = 32768             # big-table rows per window (aligned region)
ROWB = 256                # table row stride (bytes)
PAYB = 96                 # used bytes per row: 64 fp8 emb + 32 fp8 h

SUB = 64                  # samples per sub-batch
NSUBT = SPC // SUB        # 16 sub-batches
NSLOT = SUB * TS          # 6400 slots per sub
M2 = 1792                 # gather list length per window per sub
WRANK = M2 // 128         # staging ranks per window
NIDX = NBANK * M2         # 7168 merged scatter indices per sub
GRP = 4                   # samples per matmul (384 PSUM cols, 1 bank)
SUPER = 8                 # samples per PSUM super-tile (2 matmul groups)

U8 = mybir.dt.uint8
FP8 = mybir.dt.float8e4
BF16 = mybir.dt.bfloat16
F32 = mybir.dt.float32
I16 = mybir.dt.int16

_cached = {}


def _raw_gather(g, out_ap, in_ap, idxs_ap, num_idxs, elem_size, queue_num):
    """dma_gather minus the elem_size%256 assert (non-transpose, HBM src)."""
    assert idxs_ap.dtype == I16
    elem_step = in_ap.ap[0][0]
    stride_bytes = elem_step * mybir.dt.size(in_ap.dtype)
    assert stride_bytes % 256 == 0
    assert ap_utils.ap_is_contiguous(in_ap.ap[1:])
    assert ap_utils.ap_is_contiguous(out_ap.ap[1:])
    assert ap_utils.ap_is_contiguous(idxs_ap.ap[1:])
    assert in_ap.ap[-1][1] == out_ap.ap[-1][1] == elem_size
    _in = g.lower_ap_dma(in_ap, for_custom_bir_dma=True)
    return g.add_instruction(
        mybir.InstDMAGatherAnt(
            name=g.bass.get_next_instruction_name(),
            ins=[*_in, g.lower_ap(idxs_ap),
                 g.lower_val_access(g.to_reg(num_idxs))],
            outs=[g.lower_ap(out_ap)],
            transpose=False, num_idxs=num_idxs, elem_size=elem_size,
            stride_bytes_256=stride_bytes // 256, gen_mode=0,
            single_packet=False, queue_num=queue_num,
            sbuf_tokens_per_rank=0, sbuf_free_dim_per_rank=0,
            sbuf_free_dim_pad_per_rank=0, sbuf_byte_offset=0,
        ))


def build_nc():
    nc = bacc.Bacc("TRN2", target_bir_lowering=False, debug=False,
                   num_devices=NCORES, num_swdge_queues=NBANK)
    btab = nc.dram_tensor("btab", [NBANK * BROWS, ROWB], U8,
                          kind="ExternalInput")
    ig = nc.dram_tensor("ig", [NSUBT, 128, NIDX // 16], I16,
                        kind="ExternalInput")
    isc = nc.dram_tensor("isc", [NSUBT, 128, NIDX // 16], I16,
                         kind="ExternalInput")
    vmat = nc.dram_tensor("vmat", [128, TS], BF16, kind="ExternalInput")
    lam = nc.dram_tensor("lam", [TS, 1], F32, kind="ExternalInput")
    onesv = nc.dram_tensor("onesv", [128, 1], F32, kind="ExternalInput")
    linp = nc.dram_tensor("linp", [128, PAYB], BF16, kind="ExternalInput")
    bffn = nc.dram_tensor("bffn", [1, 1], F32, kind="ExternalInput")
    # Two half-tables per sub (windows 0,1 -> A; 2,3 -> B) so the two
    # scatters have no WAW conflict and run on two queues in parallel;
    # hop2 reads A then B with accum add (exact: disjoint rows on zeros).
    # +128 dump rows: scatter-list pads land there, never read back
    # (pad +0 onto a real row races its real write's RMW).
    slotb = [[nc.dram_tensor(f"slot{s}_{h}", [NSLOT + 128, ROWB], U8,
                             kind="Internal") for h in range(2)]
             for s in range(NSUBT)]
    y = nc.dram_tensor("y", [1, SPC], F32, kind="ExternalOutput")

    with TileContext(nc) as tc:
        with (
            tc.tile_pool(name="const", bufs=1) as cpool,
            tc.tile_pool(name="acc", bufs=1) as apool,
            tc.tile_pool(name="stg", bufs=6) as spool,
            tc.tile_pool(name="sel", bufs=4) as lpool,
            tc.tile_pool(name="sq", bufs=2) as qpool,
            tc.tile_pool(name="psum", bufs=2, space="PSUM") as ppool,
            tc.tile_pool(name="psuml", bufs=1, space="PSUM") as plpool,
        ):
            # index tiles first so sub-0 gathers can start ASAP
            ig_sb = cpool.tile([128, NSUBT, NIDX // 16], I16)
            nc.sync.dma_start(out=ig_sb[:],
                              in_=ig.ap().rearrange("s p n -> p s n"))
            isc_sb = cpool.tile([128, NSUBT, NIDX // 16], I16)
            nc.sync.dma_start(out=isc_sb[:],
                              in_=isc.ap().rearrange("s p n -> p s n"))
            v_sb = cpool.tile([128, TS], BF16)
            nc.sync.dma_start(out=v_sb[:], in_=vmat.ap())
            lam_sb = cpool.tile([TS, 1], F32)
            nc.sync.dma_start(out=lam_sb[:], in_=lam.ap())
            ones_sb = cpool.tile([128, 1], F32)
            nc.sync.dma_start(out=ones_sb[:], in_=onesv.ap())
            lin_sb = cpool.tile([128, PAYB], BF16)
            nc.sync.dma_start(out=lin_sb[:], in_=linp.ap())
            bffn_sb = cpool.tile([1, 1], F32)
            nc.sync.dma_start(out=bffn_sb[:], in_=bffn.ap())
            zer_sb = cpool.tile([128, NSLOT // 128, ROWB], U8)
            nc.vector.memset(zer_sb[:], 0.0)

            acc_sq = apool.tile([TS, SPC], F32)
            acc_lin = apool.tile([TS, SPC], F32)

            def emit_zero(s):
                for h in range(2):
                    nc.sync.dma_start(
                        out=slotb[s][h].ap()[0:NSLOT, :]
                        .rearrange("(p r) e -> p r e", p=128),
                        in_=zer_sb[:])

            def emit_gathers(s):
                stg = spool.tile([128, NBANK * WRANK, PAYB], U8, tag="stg")
                for k in range(NBANK):
                    r0 = k * WRANK
                    _raw_gather(
                        nc.gpsimd, stg[:, r0:r0 + WRANK, :],
                        btab.ap()[k * BROWS:(k + 1) * BROWS, 0:PAYB],
                        ig_sb[:, s, k * M2 // 16:(k + 1) * M2 // 16],
                        M2, PAYB, queue_num=k)
                return stg

            def emit_scatter(s, stg):
                half = NIDX // 2
                for h in range(2):
                    nc.gpsimd.dma_scatter_add(
                        out_ap=slotb[s][h].ap()[:, 0:PAYB],
                        in_ap=stg[:, h * 2 * WRANK:(h + 1) * 2 * WRANK, :],
                        idxs_ap=isc_sb[:, s, h * half // 16:
                                       (h + 1) * half // 16],
                        num_idxs=half, num_idxs_reg=half,
                        elem_size=PAYB, elem_step=ROWB,
                        single_packet=False,
                        queue_num=(2 * s + h) % NBANK)

            def emit_tail(s):
                sel = lpool.tile([TS, SUB, ROWB], U8, tag="sel")
                nc.scalar.dma_start(
                    out=sel[:],
                    in_=slotb[s][0].ap()[0:NSLOT, :]
                    .rearrange("(t b) e -> t b e", b=SUB))
                nc.sync.dma_start(
                    out=sel[:],
                    in_=slotb[s][1].ap()[0:NSLOT, :]
                    .rearrange("(t b) e -> t b e", b=SUB),
                    accum_op=mybir.AluOpType.add)
                sel8 = sel[:].bitcast(FP8)
                ng = SUPER // GRP
                for t in range(SUB // SUPER):
                    p = ppool.tile([TS, ng, 512], F32, space="PSUM", tag="p")
                    for g in range(ng):
                        nc.tensor.matmul(
                            out=p[:, g, 0:GRP * PAYB],
                            lhsT=v_sb[0:TS],
                            rhs=sel8[:, t * SUPER + g * GRP:
                                     t * SUPER + (g + 1) * GRP, 0:PAYB],
                            start=True, stop=True)
                    base = s * SUB + t * SUPER
                    pv = p[:, :, 0:GRP * PAYB].rearrange(
                        "p g (b e) -> p g b e", e=PAYB)
                    sq = qpool.tile([TS, ng, GRP, F], BF16, tag="sq")
                    nc.scalar.activation(
                        sq[:], pv[:, :, :, 0:F],
                        mybir.ActivationFunctionType.Square)
                    nc.vector.tensor_reduce(
                        out=acc_sq[:, base:base + SUPER].rearrange(
                            "p (g b) -> p g b", g=ng),
                        in_=sq[:],
                        axis=mybir.AxisListType.X, op=mybir.AluOpType.add)
                    lin = qpool.tile([TS, ng, GRP, PAYB], BF16, tag="lin")
                    nc.vector.tensor_tensor(
                        out=lin[:], in0=pv,
                        in1=lin_sb[0:TS]
                        .rearrange("p (a b e) -> p a b e", a=1, b=1)
                        .to_broadcast([TS, ng, GRP, PAYB]),
                        op=mybir.AluOpType.mult)
                    nc.vector.tensor_reduce(
                        out=acc_lin[:, base:base + SUPER].rearrange(
                            "p (g b) -> p g b", g=ng),
                        in_=lin[:],
                        axis=mybir.AxisListType.X, op=mybir.AluOpType.add)

            # software pipeline: scatter of sub s is emitted after the
            # gathers of sub s+2, so its staging drains are long done and
            # the Pool sequencer never parks at the queue head.
            emit_zero(0)
            emit_zero(1)
            stgs = {}
            for s in range(NSUBT):
                if s + 2 < NSUBT:
                    emit_zero(s + 2)
                stgs[s] = emit_gathers(s)
                if s >= 1:
                    emit_scatter(s - 1, stgs.pop(s - 1))
                    emit_tail(s - 1)
            emit_scatter(NSUBT - 1, stgs.pop(NSUBT - 1))
            emit_tail(NSUBT - 1)

            pl = plpool.tile([1, SPC], F32, space="PSUM")
            for h in range((SPC + 511) // 512):
                sl = slice(h * 512, min((h + 1) * 512, SPC))
                nc.tensor.matmul(out=pl[:, sl], lhsT=lam_sb[:],
                                 rhs=acc_sq[:, sl], start=True, stop=False)
                nc.tensor.matmul(out=pl[:, sl], lhsT=ones_sb[0:TS],
                                 rhs=acc_lin[:, sl], start=False, stop=True)
            y_sb = cpool.tile([1, SPC], F32)
            nc.scalar.activation(y_sb[:], pl[:],
                                 mybir.ActivationFunctionType.Sigmoid,
                                 bias=bffn_sb[:, :])
            nc.sync.dma_start(out=y.ap(), in_=y_sb[:])

    nc.compile()
    return nc


def _wrap16(flat):
    """[N] int16 list -> [128, N//16] wrapped+replicated index tile."""
    n = flat.shape[0]
    w = flat.reshape(n // 16, 16).T
    return np.tile(w, (8, 1)).astype(np.int16)


def _host_prep(x, emb, w_deep, b_deep, w_ffn, b_ffn):
    x = np.asarray(x)
    emb = np.asarray(emb, dtype=np.float32)
    w_deep = np.asarray(w_deep, dtype=np.float32)
    b_deep = np.asarray(b_deep, dtype=np.float32)
    w_ffn = np.asarray(w_ffn, dtype=np.float32).reshape(-1)
    b_ffn = np.asarray(b_ffn, dtype=np.float32).reshape(-1)

    n_deep = TS * K
    n_fm = TS * (TS - 1) // 2
    w1 = w_ffn[:n_deep].reshape(TS, K)
    w2 = w_ffn[n_deep:n_deep + n_fm].astype(np.float64)
    w3 = w_ffn[n_deep + n_fm:].reshape(TS, F)

    iu, ju = np.triu_indices(TS, k=1)
    A = np.zeros((TS, TS), dtype=np.float64)
    A[iu, ju] = w2 / 2
    A = A + A.T
    lam, V = np.linalg.eigh(A)

    fp8_np = mybir.dt.np(FP8)
    bf16_np = mybir.dt.np(BF16)

    emb8 = (emb * EMB_SCALE).astype(fp8_np)                        # [V, 64]
    hfeat = (np.maximum(emb.astype(np.float64) @ w_deep + b_deep, 0.0)
             * H_SCALE).astype(fp8_np)                             # [V, 32]
    btab = np.zeros((NBANK * BROWS, ROWB), dtype=np.uint8)
    for k in range(NBANK):
        lo = k * WINW
        n = min(WINW, VOCAB - lo)
        if n <= 0:
            break
        rows = slice(k * BROWS, k * BROWS + n)
        btab[rows, 0:F] = emb8[lo:lo + n].view(np.uint8)
        btab[rows, F:PAYB] = hfeat[lo:lo + n].view(np.uint8)

    vz = np.zeros((128, TS), dtype=bf16_np)
    vz[:TS, :] = V.astype(bf16_np)
    lam_dev = (lam / (EMB_SCALE * EMB_SCALE)).astype(np.float32).reshape(TS, 1)
    onesz = np.zeros((128, 1), dtype=np.float32)
    onesz[:TS] = 1.0
    w3t = (V.T @ w3) / EMB_SCALE                                   # [TS, 64]
    w1t = (V.T @ w1) / H_SCALE                                     # [TS, 32]
    linp = np.zeros((128, PAYB), dtype=bf16_np)
    linp[:TS, 0:F] = w3t.astype(bf16_np)
    linp[:TS, F:PAYB] = w1t.astype(bf16_np)

    shared = {
        "btab": btab, "vmat": vz, "lam": lam_dev, "onesv": onesz,
        "linp": linp, "bffn": b_ffn.reshape(1, 1).astype(np.float32),
    }

    xi = x.astype(np.int64)
    in_maps = []
    for core in range(NCORES):
        xs = xi[core * SPC:(core + 1) * SPC]                       # [SPC, TS]
        ig = np.zeros((NSUBT, 128, NIDX // 16), dtype=np.int16)
        isc = np.zeros((NSUBT, 128, NIDX // 16), dtype=np.int16)
        for s in range(NSUBT):
            xc = xs[s * SUB:(s + 1) * SUB]                         # [SUB, TS]
            win = (xc // WINW).T.reshape(-1)     # t-major [TS*SUB]
            loc = (xc - (xc // WINW) * WINW).T.reshape(-1)
            pos = np.arange(NSLOT)               # t-major slot id t*SUB+b
            gl = np.empty(NIDX, dtype=np.int64)
            sl = np.empty(NIDX, dtype=np.int64)
            for k in range(NBANK):
                msk = win == k
                n = int(msk.sum())
                assert n <= M2, n
                glk = np.full(M2, WINW, dtype=np.int64)  # pad -> zero row
                glk[:n] = loc[msk]
                # pads scatter into spread dump rows beyond the table
                slk = NSLOT + (np.arange(M2) % 128)
                slk[:n] = pos[msk]
                gl[k * M2:(k + 1) * M2] = glk
                sl[k * M2:(k + 1) * M2] = slk
            ig[s] = _wrap16(gl.astype(np.int16))
            isc[s] = _wrap16(sl.astype(np.int16))
        in_maps.append({"ig": ig, "isc": isc, **shared})
    return in_maps


def kernel(x, emb, w_deep, b_deep, w_ffn, b_ffn):
    if "nc" not in _cached:
        _cached["nc"] = build_nc()
    nc = _cached["nc"]
    in_maps = _host_prep(x, emb, w_deep, b_deep, w_ffn, b_ffn)
    trace = os.environ.get("KERNEL_TRACE", "") == "1"
    res = run_bass_kernel_spmd(nc, in_maps, core_ids=list(range(NCORES)),
                               trace=trace)
    if trace and res.exec_time_ns is not None:
        print(f"HW exec time: {res.exec_time_ns} ns")
        print(f"mean exec time: {res.mean_exec_time_ns} ns")
        if res.instructions_and_trace:
            print(f"trace: {res.instructions_and_trace[1]}")
    out = np.concatenate([res.results[c]["y"].reshape(SPC)
                          for c in range(NCORES)])
    return out.reshape(BS, 1).astype(np.float32)


# revision 17
# speedup vs baseline: 1.0638x; 1.0166x over previous
"""DeepFM forward on 8 Trainium2 NeuronCores.

Data-parallel: batch 8192 -> 1024 samples/core; tables replicated.

Math (weight-only preprocessing on host):
  logit_b = fm_b + wide_b + deep_b + b_ffn
  A = sym(w2)/2 = V diag(lam) V^T (float64 eigh); E_b = emb[x_b] [TS, F]
  P_b = V^T [E_b*32 | H_b*2048]  (fp8 rhs, one matmul per 4 samples)
  fm_b   = sum_k lam_k/32^2 sum_f P_b[k, f<64]^2
  wide_b + deep_b = <[V^T W3/32 | V^T W1/2048], P_b>_F   (DVE mult+reduce)

Gather strategy: the problem is a per-core random fetch of 102400
(sample, slot) rows x 96 B from a 100k-row table. SWDGE desc-gen runs
~7.8 ns/desc per queue, 4 queues concurrent, so total SWDGE descriptor
count is the roofline. Per sub-batch of 64 samples (6400 slots):
  1. 4 window-gathers (vocab split into 4x 25000-row windows so local
     ids fit int16; window k -> queue k) fetch the sub's slots in
     t-major order, compacted per window, into one staging tile.
  2. ONE dma_scatter_add (queue s%4, indices = each staged slot's
     t*64+b position) permutes staging into a zeroed per-sub HBM slot
     table ([6400 rows, 256 B stride], adds land on zeros; list pads
     gather the window zero-row and scatter +0 onto spread rows).
  3. One contiguous HWDGE read (100 descs x 16 KB) lands
     sel[t-partition, sample, 256B]; the matmul rhs strides 96 of 256.
No compact table, no writeback, no cross-sub barriers: each queue
streams gather gens every sub plus one merged scatter every 4th sub.
  dma_gather needs single_packet=False (>=64 descs/lane crashes the
  device otherwise); 96 B elems at 256 B stride need raw
  InstDMAGatherAnt construction (bass's %256 elem assert is
  transpose-only in ucode). -1 list tails require a matching runtime
  count register, so pads use the window zero-row instead.
"""

import os
import numpy as np

import concourse.bass as bass
import concourse.mybir as mybir
from concourse import bacc, ap_utils
from concourse.tile import TileContext
from concourse.bass_utils import run_bass_kernel_spmd

BS, TS, VOCAB, F = 8192, 100, 100000, 64
K = 32
NCORES = 8
SPC = BS // NCORES        # 1024 samples per core
EMB_SCALE = 32.0
H_SCALE = 2048.0

WINW = 25000              # vocab rows per window (balanced across queues)
NBANK = 4
BROWS = 32768             # big-table rows per window (aligned region)
ROWB = 256                # table row stride (bytes)
PAYB = 96                 # used bytes per row: 64 fp8 emb + 32 fp8 h

SUB = 64                  # samples per sub-batch
NSUBT = SPC // SUB        # 16 sub-batches
NSLOT = SUB * TS          # 6400 slots per sub
M2 = 1792                 # gather list length per window per sub
WRANK = M2 // 128         # staging ranks per window
NIDX = NBANK * M2         # 7168 merged scatter indices per sub
GRP = 4                   # samples per matmul (384 PSUM cols, 1 bank)
SUPER = 8                 # samples per PSUM super-tile (2 matmul groups)

U8 = mybir.dt.uint8
FP8 = mybir.dt.float8e4
BF16 = mybir.dt.bfloat16
F32 = mybir.dt.float32
I16 = mybir.dt.int16

_cached = {}


def _raw_gather(g, out_ap, in_ap, idxs_ap, num_idxs, elem_size, queue_num):
    """dma_gather minus the elem_size%256 assert (non-transpose, HBM src)."""
    assert idxs_ap.dtype == I16
    elem_step = in_ap.ap[0][0]
    stride_bytes = elem_step * mybir.dt.size(in_ap.dtype)
    assert stride_bytes % 256 == 0
    assert ap_utils.ap_is_contiguous(in_ap.ap[1:])
    assert ap_utils.ap_is_contiguous(out_ap.ap[1:])
    assert ap_utils.ap_is_contiguous(idxs_ap.ap[1:])
    assert in_ap.ap[-1][1] == out_ap.ap[-1][1] == elem_size
    _in = g.lower_ap_dma(in_ap, for_custom_bir_dma=True)
    return g.add_instruction(
        mybir.InstDMAGatherAnt(
            name=g.bass.get_next_instruction_name(),
            ins=[*_in, g.lower_ap(idxs_ap),
                 g.lower_val_access(g.to_reg(num_idxs))],
            outs=[g.lower_ap(out_ap)],
            transpose=False, num_idxs=num_idxs, elem_size=elem_size,
            stride_bytes_256=stride_bytes // 256, gen_mode=0,
            single_packet=False, queue_num=queue_num,
            sbuf_tokens_per_rank=0, sbuf_free_dim_per_rank=0,
            sbuf_free_dim_pad_per_rank=0, sbuf_byte_offset=0,
        ))


def build_nc():
    nc = bacc.Bacc("TRN2", target_bir_lowering=False, debug=False,
                   num_devices=NCORES, num_swdge_queues=NBANK)
    btab = nc.dram_tensor("btab", [NBANK * BROWS, ROWB], U8,
                          kind="ExternalInput")
    ig = nc.dram_tensor("ig", [NSUBT, 128, NIDX // 16], I16,
                        kind="ExternalInput")
    isc = nc.dram_tensor("isc", [NSUBT, 128, NIDX // 16], I16,
                         kind="ExternalInput")
    vmat = nc.dram_tensor("vmat", [128, TS], BF16, kind="ExternalInput")
    lam = nc.dram_tensor("lam", [TS, 1], F32, kind="ExternalInput")
    onesv = nc.dram_tensor("onesv", [128, 1], F32, kind="ExternalInput")
    linp = nc.dram_tensor("linp", [128, PAYB], BF16, kind="ExternalInput")
    bffn = nc.dram_tensor("bffn", [1, 1], F32, kind="ExternalInput")
    # +128 dump rows: scatter-list pads land there, never read back
    # (pad +0 onto a real row races its real write's RMW).
    slotb = [nc.dram_tensor(f"slot{s}", [NSLOT + 128, ROWB], U8,
                            kind="Internal")
             for s in range(NSUBT)]
    y = nc.dram_tensor("y", [1, SPC], F32, kind="ExternalOutput")

    with TileContext(nc) as tc:
        with (
            tc.tile_pool(name="const", bufs=1) as cpool,
            tc.tile_pool(name="acc", bufs=1) as apool,
            tc.tile_pool(name="stg", bufs=4) as spool,
            tc.tile_pool(name="sel", bufs=4) as lpool,
            tc.tile_pool(name="sq", bufs=2) as qpool,
            tc.tile_pool(name="psum", bufs=2, space="PSUM") as ppool,
            tc.tile_pool(name="psuml", bufs=1, space="PSUM") as plpool,
        ):
            # index tiles first so sub-0 gathers can start ASAP
            ig_sb = cpool.tile([128, NSUBT, NIDX // 16], I16)
            nc.sync.dma_start(out=ig_sb[:],
                              in_=ig.ap().rearrange("s p n -> p s n"))
            isc_sb = cpool.tile([128, NSUBT, NIDX // 16], I16)
            nc.sync.dma_start(out=isc_sb[:],
                              in_=isc.ap().rearrange("s p n -> p s n"))
            v_sb = cpool.tile([128, TS], BF16)
            nc.sync.dma_start(out=v_sb[:], in_=vmat.ap())
            lam_sb = cpool.tile([TS, 1], F32)
            nc.sync.dma_start(out=lam_sb[:], in_=lam.ap())
            ones_sb = cpool.tile([128, 1], F32)
            nc.sync.dma_start(out=ones_sb[:], in_=onesv.ap())
            lin_sb = cpool.tile([128, PAYB], BF16)
            nc.sync.dma_start(out=lin_sb[:], in_=linp.ap())
            bffn_sb = cpool.tile([1, 1], F32)
            nc.sync.dma_start(out=bffn_sb[:], in_=bffn.ap())
            zer_sb = cpool.tile([128, NSLOT // 128, ROWB], U8)
            nc.vector.memset(zer_sb[:], 0.0)

            acc_sq = apool.tile([TS, SPC], F32)
            acc_lin = apool.tile([TS, SPC], F32)

            def emit_zero(s):
                nc.sync.dma_start(
                    out=slotb[s].ap()[0:NSLOT, :]
                    .rearrange("(p r) e -> p r e", p=128),
                    in_=zer_sb[:])

            def gpair(s):
                return (0, 1) if s % 2 == 0 else (2, 3)

            def emit_gather(s, k, q):
                stg = stgs[s]
                r0 = k * WRANK
                _raw_gather(
                    nc.gpsimd, stg[:, r0:r0 + WRANK, :],
                    btab.ap()[k * BROWS:(k + 1) * BROWS, 0:PAYB],
                    ig_sb[:, s, k * M2 // 16:(k + 1) * M2 // 16],
                    M2, PAYB, queue_num=q)

            def emit_scatter_half(s, h, q):
                half = NIDX // 2
                nc.gpsimd.dma_scatter_add(
                    out_ap=slotb[s].ap()[:, 0:PAYB],
                    in_ap=stgs[s][:, h * 2 * WRANK:(h + 1) * 2 * WRANK, :],
                    idxs_ap=isc_sb[:, s, h * half // 16:(h + 1) * half // 16],
                    num_idxs=half, num_idxs_reg=half,
                    elem_size=PAYB, elem_step=ROWB,
                    single_packet=False, queue_num=q)

            def emit_tail(s):
                sel = lpool.tile([TS, SUB, ROWB], U8, tag="sel")
                eng = nc.scalar if s % 2 == 0 else nc.sync
                eng.dma_start(
                    out=sel[:],
                    in_=slotb[s].ap()[0:NSLOT, :]
                    .rearrange("(t b) e -> t b e", b=SUB))
                sel8 = sel[:].bitcast(FP8)
                ng = SUPER // GRP
                for t in range(SUB // SUPER):
                    p = ppool.tile([TS, ng, 512], F32, space="PSUM", tag="p")
                    for g in range(ng):
                        nc.tensor.matmul(
                            out=p[:, g, 0:GRP * PAYB],
                            lhsT=v_sb[0:TS],
                            rhs=sel8[:, t * SUPER + g * GRP:
                                     t * SUPER + (g + 1) * GRP, 0:PAYB],
                            start=True, stop=True)
                    base = s * SUB + t * SUPER
                    pv = p[:, :, 0:GRP * PAYB].rearrange(
                        "p g (b e) -> p g b e", e=PAYB)
                    sq = qpool.tile([TS, ng, GRP, F], BF16, tag="sq")
                    nc.scalar.activation(
                        sq[:], pv[:, :, :, 0:F],
                        mybir.ActivationFunctionType.Square)
                    nc.vector.tensor_reduce(
                        out=acc_sq[:, base:base + SUPER].rearrange(
                            "p (g b) -> p g b", g=ng),
                        in_=sq[:],
                        axis=mybir.AxisListType.X, op=mybir.AluOpType.add)
                    lin = qpool.tile([TS, ng, GRP, PAYB], BF16, tag="lin")
                    nc.vector.tensor_tensor(
                        out=lin[:], in0=pv,
                        in1=lin_sb[0:TS]
                        .rearrange("p (a b e) -> p a b e", a=1, b=1)
                        .to_broadcast([TS, ng, GRP, PAYB]),
                        op=mybir.AluOpType.mult)
                    nc.vector.tensor_reduce(
                        out=acc_lin[:, base:base + SUPER].rearrange(
                            "p (g b) -> p g b", g=ng),
                        in_=lin[:],
                        axis=mybir.AxisListType.X, op=mybir.AluOpType.add)

            # software pipeline: scatter of sub s is emitted after the
            # gathers of sub s+2, so its staging drains are long done and
            # the Pool sequencer never parks at the queue head.
            emit_zero(0)
            emit_zero(1)
            stgs = {}
            for s in range(NSUBT + 3):
                if s + 2 < NSUBT:
                    emit_zero(s + 2)
                g = gpair(s)
                sc = gpair(s + 1)
                if s < NSUBT:
                    stgs[s] = spool.tile(
                        [128, NBANK * WRANK, PAYB], U8, tag="stg",
                        name=f"stg{s}")
                    emit_gather(s, 0, g[0])
                    emit_gather(s, 2, g[1])
                if s - 2 >= 0 and s - 2 < NSUBT:
                    emit_scatter_half(s - 2, 0, sc[0])
                if s - 3 >= 0:
                    emit_scatter_half(s - 3, 1, sc[1])
                if s < NSUBT:
                    emit_gather(s, 1, g[0])
                    emit_gather(s, 3, g[1])
                if s - 3 >= 0:
                    stgs.pop(s - 3)
                    emit_tail(s - 3)

            pl = plpool.tile([1, SPC], F32, space="PSUM")
            for h in range((SPC + 511) // 512):
                sl = slice(h * 512, min((h + 1) * 512, SPC))
                nc.tensor.matmul(out=pl[:, sl], lhsT=lam_sb[:],
                                 rhs=acc_sq[:, sl], start=True, stop=False)
                nc.tensor.matmul(out=pl[:, sl], lhsT=ones_sb[0:TS],
                                 rhs=acc_lin[:, sl], start=False, stop=True)
            y_sb = cpool.tile([1, SPC], F32)
            nc.scalar.activation(y_sb[:], pl[:],
                                 mybir.ActivationFunctionType.Sigmoid,
                                 bias=bffn_sb[:, :])
            nc.sync.dma_start(out=y.ap(), in_=y_sb[:])

    nc.compile()
    return nc


def _wrap16(flat):
    """[N] int16 list -> [128, N//16] wrapped+replicated index tile."""
    n = flat.shape[0]
    w = flat.reshape(n // 16, 16).T
    return np.tile(w, (8, 1)).astype(np.int16)


def _host_prep(x, emb, w_deep, b_deep, w_ffn, b_ffn):
    x = np.asarray(x)
    emb = np.asarray(emb, dtype=np.float32)
    w_deep = np.asarray(w_deep, dtype=np.float32)
    b_deep = np.asarray(b_deep, dtype=np.float32)
    w_ffn = np.asarray(w_ffn, dtype=np.float32).reshape(-1)
    b_ffn = np.asarray(b_ffn, dtype=np.float32).reshape(-1)

    n_deep = TS * K
    n_fm = TS * (TS - 1) // 2
    w1 = w_ffn[:n_deep].reshape(TS, K)
    w2 = w_ffn[n_deep:n_deep + n_fm].astype(np.float64)
    w3 = w_ffn[n_deep + n_fm:].reshape(TS, F)

    iu, ju = np.triu_indices(TS, k=1)
    A = np.zeros((TS, TS), dtype=np.float64)
    A[iu, ju] = w2 / 2
    A = A + A.T
    lam, V = np.linalg.eigh(A)

    fp8_np = mybir.dt.np(FP8)
    bf16_np = mybir.dt.np(BF16)

    emb8 = (emb * EMB_SCALE).astype(fp8_np)                        # [V, 64]
    hfeat = (np.maximum(emb.astype(np.float64) @ w_deep + b_deep, 0.0)
             * H_SCALE).astype(fp8_np)                             # [V, 32]
    btab = np.zeros((NBANK * BROWS, ROWB), dtype=np.uint8)
    for k in range(NBANK):
        lo = k * WINW
        n = min(WINW, VOCAB - lo)
        if n <= 0:
            break
        rows = slice(k * BROWS, k * BROWS + n)
        btab[rows, 0:F] = emb8[lo:lo + n].view(np.uint8)
        btab[rows, F:PAYB] = hfeat[lo:lo + n].view(np.uint8)

    vz = np.zeros((128, TS), dtype=bf16_np)
    vz[:TS, :] = V.astype(bf16_np)
    lam_dev = (lam / (EMB_SCALE * EMB_SCALE)).astype(np.float32).reshape(TS, 1)
    onesz = np.zeros((128, 1), dtype=np.float32)
    onesz[:TS] = 1.0
    w3t = (V.T @ w3) / EMB_SCALE                                   # [TS, 64]
    w1t = (V.T @ w1) / H_SCALE                                     # [TS, 32]
    linp = np.zeros((128, PAYB), dtype=bf16_np)
    linp[:TS, 0:F] = w3t.astype(bf16_np)
    linp[:TS, F:PAYB] = w1t.astype(bf16_np)

    shared = {
        "btab": btab, "vmat": vz, "lam": lam_dev, "onesv": onesz,
        "linp": linp, "bffn": b_ffn.reshape(1, 1).astype(np.float32),
    }

    xi = x.astype(np.int64)
    in_maps = []
    for core in range(NCORES):
        xs = xi[core * SPC:(core + 1) * SPC]                       # [SPC, TS]
        ig = np.zeros((NSUBT, 128, NIDX // 16), dtype=np.int16)
        isc = np.zeros((NSUBT, 128, NIDX // 16), dtype=np.int16)
        for s in range(NSUBT):
            xc = xs[s * SUB:(s + 1) * SUB]                         # [SUB, TS]
            win = (xc // WINW).T.reshape(-1)     # t-major [TS*SUB]
            loc = (xc - (xc // WINW) * WINW).T.reshape(-1)
            pos = np.arange(NSLOT)               # t-major slot id t*SUB+b
            gl = np.empty(NIDX, dtype=np.int64)
            sl = np.empty(NIDX, dtype=np.int64)
            for k in range(NBANK):
                msk = win == k
                n = int(msk.sum())
                assert n <= M2, n
                glk = np.full(M2, WINW, dtype=np.int64)  # pad -> zero row
                glk[:n] = loc[msk]
                # pads scatter into spread dump rows beyond the table
                slk = NSLOT + (np.arange(M2) % 128)
                slk[:n] = pos[msk]
                gl[k * M2:(k + 1) * M2] = glk
                sl[k * M2:(k + 1) * M2] = slk
            ig[s] = _wrap16(gl.astype(np.int16))
            isc[s] = _wrap16(sl.astype(np.int16))
        in_maps.append({"ig": ig, "isc": isc, **shared})
    return in_maps


def kernel(x, emb, w_deep, b_deep, w_ffn, b_ffn):
    if "nc" not in _cached:
        _cached["nc"] = build_nc()
    nc = _cached["nc"]
    in_maps = _host_prep(x, emb, w_deep, b_deep, w_ffn, b_ffn)
    trace = os.environ.get("KERNEL_TRACE", "") == "1"
    res = run_bass_kernel_spmd(nc, in_maps, core_ids=list(range(NCORES)),
                               trace=trace)
    if trace and res.exec_time_ns is not None:
        print(f"HW exec time: {res.exec_time_ns} ns")
        print(f"mean exec time: {res.mean_exec_time_ns} ns")
        if res.instructions_and_trace:
            print(f"trace: {res.instructions_and_trace[1]}")
    out = np.concatenate([res.results[c]["y"].reshape(SPC)
                          for c in range(NCORES)])
    return out.reshape(BS, 1).astype(np.float32)
